# revision 16
# baseline (speedup 1.0000x reference)
"""Trainium2 Bass kernel for ragged 2x2 average-pooling merger (AvgPoolingMerger).

Reference computation per sample b:
  H2 = thw[b,1]//2, W2 = thw[b,2]//2 (both even), Hp=H2//2, Wp=W2//2,
  n_out = Hp*Wp. Output token j=(r,c) (j < n_out) is the mean of the 4
  hidden_states rows {(2r+dr)*W2 + (2c+dc), dr,dc in {0,1}} of sample b;
  tokens j >= n_out are zero. outputs_attention[b,j] = j < n_out.

Device strategy (pure data parallel, 32 samples/core, one SPMD NEFF):
  The 2x2 windows exactly tile the contiguous token prefix [0, H2*W2).
  Work unit = (sample, pooled row r, column pair c2) covering output tokens
  (r, 2*c2) and (r, 2*c2+1). Its 8 source rows form two 16KB quads of 4
  consecutive DRAM rows: top quad at row 2r*W2 + 4*c2, bottom quad at
  +W2. Per 128-unit tile:
    - 2 indirect DMA gathers (row-granular offsets, 16KB descriptors)
    - DVE: gt += gb (vertical sums), then fused (even+odd)*0.25 via
      tensor_tensor_reduce -> pooled pair [128, 2048]
    - contiguous HWDGE store: unit stream order == padded output row order
      (each sample's pooled grid stored as [Hp, Wp2] rows, Wp2 = Wp rounded
      up to even), so no scatter is needed at all.
  Host assembles the final [256,195,1024] from the per-sample padded blocks
  (invalid tokens come from the np.zeros skeleton). Pad columns/units carry
  garbage that the host never reads.
"""
import math
import os
import sys
from contextlib import ExitStack

if "/opt/trn_rl_repo" not in sys.path:
    sys.path.append("/opt/trn_rl_repo")  # fallback; sitecustomize usually provides it

import numpy as np

B, S, D = 256, 780, 1024
T_MAX = 195          # padded output tokens per sample
N_CORES = 8
BPC = B // N_CORES   # samples per core
OUT_ROWS = BPC * T_MAX        # 6240 logical output rows per core
HS_ROWS = BPC * S             # 24960 hidden rows per core
ATT_COLS = (OUT_ROWS + 127) // 128  # 49

_module_cache: dict[int, object] = {}
last_results = None  # BassKernelResults of the most recent run (for profiling)
_trace_ready = False


def _setup_trace() -> bool:
    """Install the NTFF profiling hook (missing antenv.axon_hooks shim)."""
    global _trace_ready
    if _trace_ready:
        return True
    try:
        import types

        try:
            from antenv.axon_hooks import set_axon_ntff_profile_hook
        except ImportError:
            _state = {"h": None}
            mod = types.ModuleType("antenv.axon_hooks")
            mod.set_axon_ntff_profile_hook = lambda h: _state.__setitem__("h", h)
            mod.get_axon_ntff_profile_hook = lambda: _state["h"]
            sys.modules["antenv.axon_hooks"] = mod
            import antenv

            antenv.axon_hooks = mod
            set_axon_ntff_profile_hook = mod.set_axon_ntff_profile_hook

        from trn_agent_boot.trn_boot import _ntff_profile_via_ctypes

        set_axon_ntff_profile_hook(
            _ntff_profile_via_ctypes("/opt/axon/libaxon_pjrt.so"))

        from concourse import bass_utils

        bass_utils.upload_artifacts = lambda tmpdir: f"file://{tmpdir}"
        _trace_ready = True
    except Exception as e:  # profiling is best-effort; execution must not break
        print(f"kernel: trace setup failed ({type(e).__name__}: {e})",
              file=sys.stderr)
        _trace_ready = False
    return _trace_ready


_neff_cache_installed = False


def _install_neff_cache():
    """Memoize the neuronxcc NEFF compile on disk (keyed by HLO bytes)."""
    global _neff_cache_installed
    if _neff_cache_installed:
        return
    try:
        import hashlib
        import pathlib

        from concourse import bass2jax

        cdir = pathlib.Path(os.environ.get("KERNEL_NEFF_CACHE_DIR",
                                           "/tmp/neff_cache"))
        cdir.mkdir(parents=True, exist_ok=True)
        orig_hook = bass2jax.neuronx_cc_hook

        def cached_hook(code, code_format, platform_version, file_prefix):
            try:
                key = hashlib.sha256(
                    repr((code, code_format, platform_version)).encode()
                ).hexdigest()
                path = cdir / f"{key}.bin"
                if path.exists():
                    return 0, path.read_bytes()
            except Exception:
                return orig_hook(code, code_format, platform_version, file_prefix)
            rc, data = orig_hook(code, code_format, platform_version, file_prefix)
            if rc == 0:
                try:
                    tmp = path.with_suffix(".tmp")
                    tmp.write_bytes(data)
                    tmp.rename(path)
                except Exception:
                    pass
            return rc, data

        bass2jax.neuronx_cc_hook = cached_hook
        _neff_cache_installed = True
    except Exception as e:
        print(f"kernel: neff cache setup failed ({type(e).__name__}: {e})",
              file=sys.stderr)


def _build_module(T: int):
    """Build + compile the shared SPMD Bass module for T tiles of 128 units."""
    if T in _module_cache:
        return _module_cache[T]
    from concourse import bacc, bass, mybir, tile

    nc = bacc.Bacc("TRN2", target_bir_lowering=False, debug=False,
                   num_devices=N_CORES)
    hs = nc.dram_tensor("hs", [HS_ROWS, D], mybir.dt.float32,
                        kind="ExternalInput")
    topq = nc.dram_tensor("topq", [128, T], mybir.dt.int32, kind="ExternalInput")
    botq = nc.dram_tensor("botq", [128, T], mybir.dt.int32, kind="ExternalInput")
    att_i = nc.dram_tensor("att_i", [128, ATT_COLS], mybir.dt.int32,
                           kind="ExternalInput")
    outp = nc.dram_tensor("outp", [T * 256, D], mybir.dt.float32,
                          kind="ExternalOutput")
    att_o = nc.dram_tensor("att_o", [128, ATT_COLS], mybir.dt.int32,
                           kind="ExternalOutput")

    with tile.TileContext(nc) as tc, ExitStack() as ctx:
        idxp = ctx.enter_context(tc.tile_pool(name="idx", bufs=1))
        tp = ctx.enter_context(tc.tile_pool(name="gt", bufs=5))
        bp = ctx.enter_context(tc.tile_pool(name="gb", bufs=5))

        top_sb = idxp.tile([128, T], mybir.dt.int32)
        bot_sb = idxp.tile([128, T], mybir.dt.int32)
        at_sb = idxp.tile([128, ATT_COLS], mybir.dt.int32)
        acc = idxp.tile([128, 1], mybir.dt.float32)  # dummy reduce target
        nc.sync.dma_start(out=top_sb[:], in_=topq[:])
        nc.sync.dma_start(out=bot_sb[:], in_=botq[:])
        nc.sync.dma_start(out=at_sb[:], in_=att_i[:])
        nc.sync.dma_start(out=att_o[:], in_=at_sb[:])

        for i in range(T):
            gt = tp.tile([128, 4096], mybir.dt.float32)
            nc.gpsimd.indirect_dma_start(
                out=gt[:], out_offset=None, in_=hs[:],
                in_offset=bass.IndirectOffsetOnAxis(ap=top_sb[:, i:i + 1], axis=0),
            )
            gb = bp.tile([128, 4096], mybir.dt.float32)
            nc.gpsimd.indirect_dma_start(
                out=gb[:], out_offset=None, in_=hs[:],
                in_offset=bass.IndirectOffsetOnAxis(ap=bot_sb[:, i:i + 1], axis=0),
            )
            nc.vector.tensor_add(gt[:], gt[:], gb[:])  # vertical window sums
            gv = gt[:].rearrange("p (a b d) -> p a b d", b=2, d=D)
            p = gb[:, 0:2048]  # reuse gb's first half for the pooled pair
            nc.vector.tensor_add(p.rearrange("p (a d) -> p a d", d=D),
                                 gv[:, :, 0, :], gv[:, :, 1, :])
            nc.vector.tensor_scalar_mul(p, p, 0.25)
            nc.sync.dma_start(
                out=outp[256 * i:256 * (i + 1), :].rearrange(
                    "(p k) d -> p (k d)", k=2),
                in_=p,
            )
    nc.compile()
    _module_cache[T] = nc
    return nc


def _pack_stream(arr: np.ndarray, T: int) -> np.ndarray:
    """Stream unit s -> SBUF [s % 128, s // 128] (tile i covers s in [128i, 128i+128))."""
    return np.ascontiguousarray(arr.reshape(T, 128).T)


def kernel(hidden_states, attention_mask, image_grid_thw):
    global last_results
    hs_np = np.ascontiguousarray(np.asarray(hidden_states), dtype=np.float32)
    thw = np.asarray(image_grid_thw)
    W2 = thw[:, 2] // 2
    Hp = (thw[:, 1] // 2) // 2
    Wp = W2 // 2
    n_out = (Hp * Wp).astype(np.int64)
    Wp2 = Wp + (Wp & 1)           # Wp rounded up to even
    units = (Hp * (Wp2 // 2)).astype(np.int64)  # work units per sample

    # Greedy balance: 32 samples per core, minimizing max total units.
    order = np.argsort(-units, kind="stable")
    loads = [0] * N_CORES
    counts = [0] * N_CORES
    assign = [[] for _ in range(N_CORES)]
    for b in order:
        c = min((c for c in range(N_CORES) if counts[c] < BPC),
                key=lambda c: loads[c])
        assign[c].append(int(b))
        counts[c] += 1
        loads[c] += int(units[b])
    T = max(1, math.ceil(max(loads) / 128))

    nc = _build_module(T)
    from concourse.bass_utils import run_bass_kernel_spmd

    in_maps = []
    for c in range(N_CORES):
        ids = assign[c]
        topq = np.zeros(T * 128, np.int32)
        botq = np.zeros(T * 128, np.int32)
        att = np.zeros(128 * ATT_COLS, np.int32)
        pos = 0
        for bl, b in enumerate(ids):
            w2, hp, no = int(W2[b]), int(Hp[b]), int(n_out[b])
            nu = int(units[b])
            tops = (bl * S + 2 * np.arange(hp)[:, None] * w2
                    + 4 * np.arange(nu // hp)[None, :]).ravel()
            topq[pos:pos + nu] = tops
            botq[pos:pos + nu] = tops + w2
            att[bl * T_MAX: bl * T_MAX + no] = 1
            pos += nu
        in_maps.append({
            "hs": hs_np[ids].reshape(HS_ROWS, D),
            "topq": _pack_stream(topq, T),
            "botq": _pack_stream(botq, T),
            "att_i": att.reshape(128, ATT_COLS),
        })

    _install_neff_cache()
    trace = bool(os.environ.get("KERNEL_TRACE")) and _setup_trace()
    last_results = run_bass_kernel_spmd(nc, in_maps, core_ids=list(range(N_CORES)),
                                        trace=trace)

    out_full = np.zeros((B, T_MAX, D), np.float32)
    att_full = np.zeros((B, T_MAX), np.asarray(attention_mask).dtype)
    for c in range(N_CORES):
        r = last_results.results[c]
        op = r["outp"]
        base = 0
        for bl, b in enumerate(assign[c]):
            hp, wp, wp2, no = int(Hp[b]), int(Wp[b]), int(Wp2[b]), int(n_out[b])
            block = op[base:base + hp * wp2].reshape(hp, wp2, D)[:, :wp, :]
            out_full[b, :no] = block.reshape(no, D)
            base += hp * wp2
        att_full[assign[c]] = r["att_o"].reshape(-1)[:OUT_ROWS].reshape(BPC, T_MAX)
    return out_full, att_full


# revision 20
# speedup vs baseline: 1.0295x; 1.0295x over previous
"""Trainium2 Bass kernel for ragged 2x2 average-pooling merger (AvgPoolingMerger).

Reference computation per sample b:
  H2 = thw[b,1]//2, W2 = thw[b,2]//2 (both even), Hp=H2//2, Wp=W2//2,
  n_out = Hp*Wp. Output token j=(r,c) (j < n_out) is the mean of the 4
  hidden_states rows {(2r+dr)*W2 + (2c+dc), dr,dc in {0,1}} of sample b;
  tokens j >= n_out are zero. outputs_attention[b,j] = j < n_out.

Device strategy (pure data parallel, 32 samples/core, one SPMD NEFF):
  The 2x2 windows exactly tile the contiguous token prefix [0, H2*W2).
  Work unit = (sample, pooled row r, column pair c2) covering output tokens
  (r, 2*c2) and (r, 2*c2+1). Its 8 source rows form two 16KB quads of 4
  consecutive DRAM rows: top quad at row 2r*W2 + 4*c2, bottom quad at
  +W2. Per 128-unit tile:
    - 2 indirect DMA gathers (row-granular offsets, 16KB descriptors; quad
      descriptors keep SWDGE Q7 descriptor generation off the critical path)
    - DVE, split into two independent half-pipelines to shorten the drain
      tail: vertical add (gt += gb), horizontal pair add, *0.25
    - contiguous/strided HWDGE stores: unit stream order == padded output
      row order (each sample's pooled grid stored as [Hp, Wp2] rows, Wp2 =
      Wp rounded up to even), so no scatter and no Q7 work for writes.
  Host assembles the final [256,195,1024] from the per-sample padded blocks
  (invalid tokens come from the np.zeros skeleton). Pad columns/units carry
  garbage that the host never reads.
"""
import math
import os
import sys
from contextlib import ExitStack

if "/opt/trn_rl_repo" not in sys.path:
    sys.path.append("/opt/trn_rl_repo")  # fallback; sitecustomize usually provides it

import numpy as np

B, S, D = 256, 780, 1024
T_MAX = 195          # padded output tokens per sample
N_CORES = 8
BPC = B // N_CORES   # samples per core
OUT_ROWS = BPC * T_MAX        # 6240 logical output rows per core
HS_ROWS = BPC * S             # 24960 hidden rows per core
ATT_COLS = (OUT_ROWS + 127) // 128  # 49

_module_cache: dict[int, object] = {}
last_results = None  # BassKernelResults of the most recent run (for profiling)
_trace_ready = False


def _setup_trace() -> bool:
    """Install the NTFF profiling hook (missing antenv.axon_hooks shim)."""
    global _trace_ready
    if _trace_ready:
        return True
    try:
        import types

        try:
            from antenv.axon_hooks import set_axon_ntff_profile_hook
        except ImportError:
            _state = {"h": None}
            mod = types.ModuleType("antenv.axon_hooks")
            mod.set_axon_ntff_profile_hook = lambda h: _state.__setitem__("h", h)
            mod.get_axon_ntff_profile_hook = lambda: _state["h"]
            sys.modules["antenv.axon_hooks"] = mod
            import antenv

            antenv.axon_hooks = mod
            set_axon_ntff_profile_hook = mod.set_axon_ntff_profile_hook

        from trn_agent_boot.trn_boot import _ntff_profile_via_ctypes

        set_axon_ntff_profile_hook(
            _ntff_profile_via_ctypes("/opt/axon/libaxon_pjrt.so"))

        from concourse import bass_utils

        bass_utils.upload_artifacts = lambda tmpdir: f"file://{tmpdir}"
        _trace_ready = True
    except Exception as e:  # profiling is best-effort; execution must not break
        print(f"kernel: trace setup failed ({type(e).__name__}: {e})",
              file=sys.stderr)
        _trace_ready = False
    return _trace_ready


_neff_cache_installed = False


def _install_neff_cache():
    """Memoize the neuronxcc NEFF compile on disk (keyed by HLO bytes)."""
    global _neff_cache_installed
    if _neff_cache_installed:
        return
    try:
        import hashlib
        import pathlib

        from concourse import bass2jax

        cdir = pathlib.Path(os.environ.get("KERNEL_NEFF_CACHE_DIR",
                                           "/tmp/neff_cache"))
        cdir.mkdir(parents=True, exist_ok=True)
        orig_hook = bass2jax.neuronx_cc_hook

        def cached_hook(code, code_format, platform_version, file_prefix):
            try:
                key = hashlib.sha256(
                    repr((code, code_format, platform_version)).encode()
                ).hexdigest()
                path = cdir / f"{key}.bin"
                if path.exists():
                    return 0, path.read_bytes()
            except Exception:
                return orig_hook(code, code_format, platform_version, file_prefix)
            rc, data = orig_hook(code, code_format, platform_version, file_prefix)
            if rc == 0:
                try:
                    tmp = path.with_suffix(".tmp")
                    tmp.write_bytes(data)
                    tmp.rename(path)
                except Exception:
                    pass
            return rc, data

        bass2jax.neuronx_cc_hook = cached_hook
        _neff_cache_installed = True
    except Exception as e:
        print(f"kernel: neff cache setup failed ({type(e).__name__}: {e})",
              file=sys.stderr)


def _build_module(T: int):
    """Build + compile the shared SPMD Bass module for T tiles of 128 units."""
    if T in _module_cache:
        return _module_cache[T]
    from concourse import bacc, bass, mybir, tile

    nc = bacc.Bacc("TRN2", target_bir_lowering=False, debug=False,
                   num_devices=N_CORES)
    hs = nc.dram_tensor("hs", [HS_ROWS, D], mybir.dt.float32,
                        kind="ExternalInput")
    topq = nc.dram_tensor("topq", [128, T], mybir.dt.int32, kind="ExternalInput")
    botq = nc.dram_tensor("botq", [128, T], mybir.dt.int32, kind="ExternalInput")
    att_i = nc.dram_tensor("att_i", [128, ATT_COLS], mybir.dt.int32,
                           kind="ExternalInput")
    outp = nc.dram_tensor("outp", [T * 256, D], mybir.dt.float32,
                          kind="ExternalOutput")
    att_o = nc.dram_tensor("att_o", [128, ATT_COLS], mybir.dt.int32,
                           kind="ExternalOutput")

    with tile.TileContext(nc) as tc, ExitStack() as ctx:
        idxp = ctx.enter_context(tc.tile_pool(name="idx", bufs=1))
        tp = ctx.enter_context(tc.tile_pool(name="gt", bufs=5))
        bp = ctx.enter_context(tc.tile_pool(name="gb", bufs=5))

        top_sb = idxp.tile([128, T], mybir.dt.int32)
        bot_sb = idxp.tile([128, T], mybir.dt.int32)
        at_sb = idxp.tile([128, ATT_COLS], mybir.dt.int32)
        nc.sync.dma_start(out=top_sb[:], in_=topq[:])
        nc.sync.dma_start(out=bot_sb[:], in_=botq[:])
        nc.sync.dma_start(out=at_sb[:], in_=att_i[:])
        nc.sync.dma_start(out=att_o[:], in_=at_sb[:])

        for i in range(T):
            gt = tp.tile([128, 4096], mybir.dt.float32)
            nc.gpsimd.indirect_dma_start(
                out=gt[:], out_offset=None, in_=hs[:],
                in_offset=bass.IndirectOffsetOnAxis(ap=top_sb[:, i:i + 1], axis=0),
            )
            gb = bp.tile([128, 4096], mybir.dt.float32)
            nc.gpsimd.indirect_dma_start(
                out=gb[:], out_offset=None, in_=hs[:],
                in_offset=bass.IndirectOffsetOnAxis(ap=bot_sb[:, i:i + 1], axis=0),
            )
            # two independent half-pipelines: [S0|S1] -> row 2u, [S2|S3] -> row 2u+1
            for h in range(2):
                sl = slice(2048 * h, 2048 * (h + 1))
                nc.vector.tensor_add(gt[:, sl], gt[:, sl], gb[:, sl])
                ph = gb[:, 1024 * h:1024 * (h + 1)]  # reuse gb space
                nc.vector.tensor_add(ph, gt[:, 2048 * h:2048 * h + 1024],
                                     gt[:, 2048 * h + 1024:2048 * (h + 1)])
                nc.vector.tensor_scalar_mul(ph, ph, 0.25)
                # rows 256i+2k+h for k in [0,128): stride 2 rows
                orows = outp[256 * i:256 * (i + 1), :].rearrange(
                    "(p two) d -> p two d", two=2)[:, h, :]
                nc.sync.dma_start(out=orows, in_=ph)
    nc.compile()
    _module_cache[T] = nc
    return nc


def _pack_stream(arr: np.ndarray, T: int) -> np.ndarray:
    """Stream unit s -> SBUF [s % 128, s // 128] (tile i covers s in [128i, 128i+128))."""
    return np.ascontiguousarray(arr.reshape(T, 128).T)


def kernel(hidden_states, attention_mask, image_grid_thw):
    global last_results
    hs_np = np.ascontiguousarray(np.asarray(hidden_states), dtype=np.float32)
    thw = np.asarray(image_grid_thw)
    W2 = thw[:, 2] // 2
    Hp = (thw[:, 1] // 2) // 2
    Wp = W2 // 2
    n_out = (Hp * Wp).astype(np.int64)
    Wp2 = Wp + (Wp & 1)           # Wp rounded up to even
    units = (Hp * (Wp2 // 2)).astype(np.int64)  # work units per sample

    # Greedy balance: 32 samples per core, minimizing max total units.
    order = np.argsort(-units, kind="stable")
    loads = [0] * N_CORES
    counts = [0] * N_CORES
    assign = [[] for _ in range(N_CORES)]
    for b in order:
        c = min((c for c in range(N_CORES) if counts[c] < BPC),
                key=lambda c: loads[c])
        assign[c].append(int(b))
        counts[c] += 1
        loads[c] += int(units[b])
    T = max(1, math.ceil(max(loads) / 128))

    nc = _build_module(T)
    from concourse.bass_utils import run_bass_kernel_spmd

    in_maps = []
    for c in range(N_CORES):
        ids = assign[c]
        topq = np.zeros(T * 128, np.int32)
        botq = np.zeros(T * 128, np.int32)
        att = np.zeros(128 * ATT_COLS, np.int32)
        pos = 0
        for bl, b in enumerate(ids):
            w2, hp, no = int(W2[b]), int(Hp[b]), int(n_out[b])
            nu = int(units[b])
            tops = (bl * S + 2 * np.arange(hp)[:, None] * w2
                    + 4 * np.arange(nu // hp)[None, :]).ravel()
            topq[pos:pos + nu] = tops
            botq[pos:pos + nu] = tops + w2
            att[bl * T_MAX: bl * T_MAX + no] = 1
            pos += nu
        in_maps.append({
            "hs": hs_np[ids].reshape(HS_ROWS, D),
            "topq": _pack_stream(topq, T),
            "botq": _pack_stream(botq, T),
            "att_i": att.reshape(128, ATT_COLS),
        })

    _install_neff_cache()
    trace = bool(os.environ.get("KERNEL_TRACE")) and _setup_trace()
    last_results = run_bass_kernel_spmd(nc, in_maps, core_ids=list(range(N_CORES)),
                                        trace=trace)

    out_full = np.zeros((B, T_MAX, D), np.float32)
    att_full = np.zeros((B, T_MAX), np.asarray(attention_mask).dtype)
    for c in range(N_CORES):
        r = last_results.results[c]
        op = r["outp"]
        base = 0
        for bl, b in enumerate(assign[c]):
            hp, wp, wp2, no = int(Hp[b]), int(Wp[b]), int(Wp2[b]), int(n_out[b])
            block = op[base:base + hp * wp2].reshape(hp, wp2, D)[:, :wp, :]
            out_full[b, :no] = block.reshape(no, D)
            base += hp * wp2
        att_full[assign[c]] = r["att_o"].reshape(-1)[:OUT_ROWS].reshape(BPC, T_MAX)
    return out_full, att_full


# revision 22
# speedup vs baseline: 1.1983x; 1.1640x over previous
"""Trainium2 Bass kernel for ragged 2x2 average-pooling merger (AvgPoolingMerger).

Reference computation per sample b:
  H2 = thw[b,1]//2, W2 = thw[b,2]//2 (both even), Hp=H2//2, Wp=W2//2,
  n_out = Hp*Wp. Output token j=(r,c) (j < n_out) is the mean of the 4
  hidden_states rows {(2r+dr)*W2 + (2c+dc), dr,dc in {0,1}} of sample b;
  tokens j >= n_out are zero. outputs_attention[b,j] = j < n_out.

Device strategy (pure data parallel, 32 samples/core, one SPMD NEFF):
  The 2x2 windows exactly tile the contiguous token prefix [0, H2*W2).
  Work unit = (sample, pooled row r, column pair c2) covering output tokens
  (r, 2*c2) and (r, 2*c2+1). Its 8 source rows form two 16KB quads of 4
  consecutive DRAM rows: top quad at row 2r*W2 + 4*c2, bottom quad at
  +W2. Per 128-unit tile:
    - 2 indirect DMA gathers (row-granular offsets, 16KB descriptors; quad
      descriptors keep SWDGE Q7 descriptor generation off the critical path)
    - DVE, split into two independent half-pipelines to shorten the drain
      tail: vertical add (gt += gb), horizontal pair add, *0.25
    - contiguous/strided HWDGE stores: unit stream order == padded output
      row order (each sample's pooled grid stored as [Hp, Wp2] rows, Wp2 =
      Wp rounded up to even), so no scatter and no Q7 work for writes.
  Host assembles the final [256,195,1024] from the per-sample padded blocks
  (invalid tokens come from the np.zeros skeleton). Pad columns/units carry
  garbage that the host never reads.
"""
import math
import os
import sys
from contextlib import ExitStack

if "/opt/trn_rl_repo" not in sys.path:
    sys.path.append("/opt/trn_rl_repo")  # fallback; sitecustomize usually provides it

import numpy as np

B, S, D = 256, 780, 1024
T_MAX = 195          # padded output tokens per sample
N_CORES = 8
BPC = B // N_CORES   # samples per core
OUT_ROWS = BPC * T_MAX        # 6240 logical output rows per core
HS_ROWS = BPC * S             # 24960 hidden rows per core
ATT_COLS = (OUT_ROWS + 127) // 128  # 49

_module_cache: dict[int, object] = {}
last_results = None  # BassKernelResults of the most recent run (for profiling)
_trace_ready = False


def _setup_trace() -> bool:
    """Install the NTFF profiling hook (missing antenv.axon_hooks shim)."""
    global _trace_ready
    if _trace_ready:
        return True
    try:
        import types

        try:
            from antenv.axon_hooks import set_axon_ntff_profile_hook
        except ImportError:
            _state = {"h": None}
            mod = types.ModuleType("antenv.axon_hooks")
            mod.set_axon_ntff_profile_hook = lambda h: _state.__setitem__("h", h)
            mod.get_axon_ntff_profile_hook = lambda: _state["h"]
            sys.modules["antenv.axon_hooks"] = mod
            import antenv

            antenv.axon_hooks = mod
            set_axon_ntff_profile_hook = mod.set_axon_ntff_profile_hook

        from trn_agent_boot.trn_boot import _ntff_profile_via_ctypes

        set_axon_ntff_profile_hook(
            _ntff_profile_via_ctypes("/opt/axon/libaxon_pjrt.so"))

        from concourse import bass_utils

        bass_utils.upload_artifacts = lambda tmpdir: f"file://{tmpdir}"
        _trace_ready = True
    except Exception as e:  # profiling is best-effort; execution must not break
        print(f"kernel: trace setup failed ({type(e).__name__}: {e})",
              file=sys.stderr)
        _trace_ready = False
    return _trace_ready


_neff_cache_installed = False


def _install_neff_cache():
    """Memoize the neuronxcc NEFF compile on disk (keyed by HLO bytes)."""
    global _neff_cache_installed
    if _neff_cache_installed:
        return
    try:
        import hashlib
        import pathlib

        from concourse import bass2jax

        cdir = pathlib.Path(os.environ.get("KERNEL_NEFF_CACHE_DIR",
                                           "/tmp/neff_cache"))
        cdir.mkdir(parents=True, exist_ok=True)
        orig_hook = bass2jax.neuronx_cc_hook

        def cached_hook(code, code_format, platform_version, file_prefix):
            try:
                key = hashlib.sha256(
                    repr((code, code_format, platform_version)).encode()
                ).hexdigest()
                path = cdir / f"{key}.bin"
                if path.exists():
                    return 0, path.read_bytes()
            except Exception:
                return orig_hook(code, code_format, platform_version, file_prefix)
            rc, data = orig_hook(code, code_format, platform_version, file_prefix)
            if rc == 0:
                try:
                    tmp = path.with_suffix(".tmp")
                    tmp.write_bytes(data)
                    tmp.rename(path)
                except Exception:
                    pass
            return rc, data

        bass2jax.neuronx_cc_hook = cached_hook
        _neff_cache_installed = True
    except Exception as e:
        print(f"kernel: neff cache setup failed ({type(e).__name__}: {e})",
              file=sys.stderr)


def _build_module(T: int):
    """Build + compile the shared SPMD Bass module for T tiles of 128 units."""
    if T in _module_cache:
        return _module_cache[T]
    from concourse import bacc, bass, mybir, tile

    nc = bacc.Bacc("TRN2", target_bir_lowering=False, debug=False,
                   num_devices=N_CORES)
    hs = nc.dram_tensor("hs", [HS_ROWS, D], mybir.dt.float32,
                        kind="ExternalInput")
    topq = nc.dram_tensor("topq", [128, T], mybir.dt.int32, kind="ExternalInput")
    botq = nc.dram_tensor("botq", [128, T], mybir.dt.int32, kind="ExternalInput")
    att_i = nc.dram_tensor("att_i", [128, ATT_COLS], mybir.dt.int32,
                           kind="ExternalInput")
    outp = nc.dram_tensor("outp", [T * 256, D], mybir.dt.float32,
                          kind="ExternalOutput")
    att_o = nc.dram_tensor("att_o", [128, ATT_COLS], mybir.dt.int32,
                           kind="ExternalOutput")

    with tile.TileContext(nc) as tc, ExitStack() as ctx:
        idxp = ctx.enter_context(tc.tile_pool(name="idx", bufs=1))
        tp = ctx.enter_context(tc.tile_pool(name="gt", bufs=5))
        bp = ctx.enter_context(tc.tile_pool(name="gb", bufs=5))

        top_sb = idxp.tile([128, T], mybir.dt.int32)
        bot_sb = idxp.tile([128, T], mybir.dt.int32)
        at_sb = idxp.tile([128, ATT_COLS], mybir.dt.int32)
        nc.sync.dma_start(out=top_sb[:], in_=topq[:])
        nc.sync.dma_start(out=bot_sb[:], in_=botq[:])
        nc.sync.dma_start(out=at_sb[:], in_=att_i[:])
        nc.sync.dma_start(out=att_o[:], in_=at_sb[:])

        for i in range(T):
            gt = tp.tile([128, 4096], mybir.dt.float32)
            nc.gpsimd.indirect_dma_start(
                out=gt[:], out_offset=None, in_=hs[:],
                in_offset=bass.IndirectOffsetOnAxis(ap=top_sb[:, i:i + 1], axis=0),
            )
            gb = bp.tile([128, 4096], mybir.dt.float32)
            nc.gpsimd.indirect_dma_start(
                out=gb[:], out_offset=None, in_=hs[:],
                in_offset=bass.IndirectOffsetOnAxis(ap=bot_sb[:, i:i + 1], axis=0),
            )
            # two independent half-pipelines: [S0|S1] -> row 2u, [S2|S3] -> row 2u+1
            for h in range(2):
                sl = slice(2048 * h, 2048 * (h + 1))
                nc.vector.tensor_add(gt[:, sl], gt[:, sl], gb[:, sl])
                ph = gb[:, 1024 * h:1024 * (h + 1)]  # reuse gb space
                nc.vector.tensor_add(ph, gt[:, 2048 * h:2048 * h + 1024],
                                     gt[:, 2048 * h + 1024:2048 * (h + 1)])
                nc.vector.tensor_scalar_mul(ph, ph, 0.25)
                # rows 256i+2k+h for k in [0,128): stride 2 rows
                orows = outp[256 * i:256 * (i + 1), :].rearrange(
                    "(p two) d -> p two d", two=2)[:, h, :]
                nc.sync.dma_start(out=orows, in_=ph)
    nc.compile()
    _module_cache[T] = nc
    return nc


def _pack_stream(arr: np.ndarray, T: int) -> np.ndarray:
    """Stream unit s -> SBUF [s % 128, s // 128] (tile i covers s in [128i, 128i+128))."""
    return np.ascontiguousarray(arr.reshape(T, 128).T)


def kernel(hidden_states, attention_mask, image_grid_thw):
    global last_results
    hs_np = np.ascontiguousarray(np.asarray(hidden_states), dtype=np.float32)
    thw = np.asarray(image_grid_thw)
    W2 = thw[:, 2] // 2
    Hp = (thw[:, 1] // 2) // 2
    Wp = W2 // 2
    n_out = (Hp * Wp).astype(np.int64)
    Wp2 = Wp + (Wp & 1)           # Wp rounded up to even
    units = (Hp * (Wp2 // 2)).astype(np.int64)  # work units per sample

    # Greedy balance: 32 samples per core, minimizing max total units.
    order = np.argsort(-units, kind="stable")
    loads = [0] * N_CORES
    counts = [0] * N_CORES
    assign = [[] for _ in range(N_CORES)]
    for b in order:
        c = min((c for c in range(N_CORES) if counts[c] < BPC),
                key=lambda c: loads[c])
        assign[c].append(int(b))
        counts[c] += 1
        loads[c] += int(units[b])
    T = max(1, math.ceil(max(loads) / 128))

    nc = _build_module(T)
    from concourse.bass_utils import run_bass_kernel_spmd

    in_maps = []
    for c in range(N_CORES):
        ids = assign[c]
        topq = np.zeros(T * 128, np.int32)
        botq = np.zeros(T * 128, np.int32)
        att = np.zeros(128 * ATT_COLS, np.int32)
        pos = 0
        for bl, b in enumerate(ids):
            w2, hp, no = int(W2[b]), int(Hp[b]), int(n_out[b])
            nu = int(units[b])
            tops = (bl * S + 2 * np.arange(hp)[:, None] * w2
                    + 4 * np.arange(nu // hp)[None, :]).ravel()
            topq[pos:pos + nu] = tops
            botq[pos:pos + nu] = tops + w2
            att[bl * T_MAX: bl * T_MAX + no] = 1
            pos += nu
        in_maps.append({
            "hs": hs_np[ids].reshape(HS_ROWS, D),
            "topq": _pack_stream(topq, T),
            "botq": _pack_stream(botq, T),
            "att_i": att.reshape(128, ATT_COLS),
        })

    _install_neff_cache()
    trace = bool(os.environ.get("KERNEL_TRACE")) and _setup_trace()
    last_results = run_bass_kernel_spmd(nc, in_maps, core_ids=list(range(N_CORES)),
                                        trace=trace)

    out_full = np.zeros((B, T_MAX, D), np.float32)
    att_full = np.zeros((B, T_MAX), np.asarray(attention_mask).dtype)
    for c in range(N_CORES):
        r = last_results.results[c]
        op = r["outp"]
        base = 0
        for bl, b in enumerate(assign[c]):
            hp, wp, wp2, no = int(Hp[b]), int(Wp[b]), int(Wp2[b]), int(n_out[b])
            block = op[base:base + hp * wp2].reshape(hp, wp2, D)[:, :wp, :]
            out_full[b, :no] = block.reshape(no, D)
            base += hp * wp2
        att_full[assign[c]] = r["att_o"].reshape(-1)[:OUT_ROWS].reshape(BPC, T_MAX)
    return out_full, att_full
